# revision 1
# baseline (speedup 1.0000x reference)
"""MultiHeadedAttention Trainium2 kernel (8-core SPMD, data-parallel).

Sharding: 8 cores = (batch b in 0..3) x (query half in 0..1). Each core
computes out[b, half*1024:(half+1)*1024, :] independently - no collectives.

Per-core dataflow (all "T" = transposed layouts, contraction dim on partitions):
  - inputs cast fp32->bf16 during DMA (SWDGE), staged to DRAM, transposed
    back into SBUF via xbar DMA transpose in 512-col slices
  - projections (bf16 matmuls, fp32 psum): qT/kT [d_head, S] per head-pair,
    v natural [Sk, d] with a trailing ones column per head (Z trick)
  - scores^T [Sk-tile, Sq-slice] = kT.T @ qT per head (K=64, two heads
    row-tiled concurrently); exp on ScalarE (scale 1/8 folded into Wq/bq,
    no max-subtraction needed: |s/8| <~ 2 by construction); mask applied
    as bf16 multiply with maskT (staged transpose of int mask)
  - PV: psum rows 0..63 = sum_j v^T p, row 64 = Z (ones col); finalize:
    PE-broadcast Z, reciprocal_approx_fast, multiply, SBUF->SBUF DMA hop
    into head-pair layout xattnT [dm, Sq]
  - out = xattnT.T @ WoT + R where R = bo + bv@WoT (PE-broadcast), fp32
"""
import numpy as np
import ml_dtypes

import concourse.bass as bass
import concourse.mybir as mybir
import concourse.tile as tile
from concourse import bacc
from concourse.bass_utils import run_bass_kernel_spmd

F32 = mybir.dt.float32
BF16 = mybir.dt.bfloat16
I32 = mybir.dt.int32
AF = mybir.ActivationFunctionType
ALU = mybir.AluOpType

N_CORES = 8
DK = 64


def slices(total, chunk):
    return [(s, min(chunk, total - s)) for s in range(0, total, chunk)]


class Cfg:
    def __init__(self, SQ=1024, SK=2048, DM=1024, H=16, max_stage=5):
        assert DM % 128 == 0 and SK % 128 == 0 and SQ % 128 == 0 and H % 2 == 0
        self.SQ, self.SK, self.DM, self.H = SQ, SK, DM, H
        self.KT = DM // 128          # dm contraction chunks
        self.HP = H // 2             # head pairs
        self.NJ = SK // 128          # Sk tiles
        self.SQS = min(1024, SQ)     # attention Sq slice width (2 psum banks)
        self.max_stage = max_stage   # debug: truncate kernel after stage N
        assert SQ % self.SQS == 0
        assert H * DK == DM


def emit_kernel(tc, cfg, io):
    nc = tc.nc
    C = cfg
    xq, xk, xv, msk = io["xq"], io["xk"], io["xv"], io["mask"]
    w_dram = {"q": io["wqt"], "k": io["wkt"], "v": io["wvt"], "o": io["wot"]}
    bql, bkl, bvl, bo_row = io["bql"], io["bkl"], io["bvl"], io["bo_row"]
    out = io["out"]

    pools = {}

    def open_pool(name, bufs=1, space="SBUF"):
        pools[name] = tc.alloc_tile_pool(name=name, bufs=bufs, space=space)
        return pools[name]

    persist = open_pool("persist", 1)
    dram = open_pool("dram", 1, space="DRAM")
    # 8 banks: "s" 2 slots x 2 banks + "pv" 2 slots x 2 banks; proj/zb/R/
    # outproj psums all share the "s" slots
    ps_s = open_pool("ps_s", 2, space="PSUM")
    ps_pv = open_pool("ps_pv", 2, space="PSUM")
    staging = open_pool("staging", 1)
    wv_pool = open_pool("wv", 1)
    xv_pool = open_pool("xv", 1)

    # ---------------- persistent tiles ----------------
    qT_sb = persist.tile([128, C.HP * C.SQ], BF16, name="qT_sb")
    kT_sb = persist.tile([128, C.HP * C.SK], BF16, name="kT_sb")
    v_sb = persist.tile([128, C.NJ * C.H * 65], BF16, name="v_sb")
    xattnT_sb = persist.tile([128, C.HP * C.SQ], BF16, name="xattnT_sb")
    maskT_sb = persist.tile([128, C.NJ * C.SQ], BF16, name="maskT_sb")
    R_sb = persist.tile([128, C.DM], F32, name="R_sb")
    bql_sb = persist.tile([128, C.HP], F32, name="bql_sb")
    bkl_sb = persist.tile([128, C.HP], F32, name="bkl_sb")
    bvl_sb = persist.tile([128, C.KT], BF16, name="bvl_sb")
    bo_sb = persist.tile([1, C.DM], F32, name="bo_sb")
    onesf_sb = persist.tile([65, 128], F32, name="onesf_sb")
    Rrow_sb = persist.tile([1, C.DM], F32, name="Rrow_sb")

    nc.sync.dma_start(bql_sb[:], bql[:])
    nc.sync.dma_start(bkl_sb[:], bkl[:])
    nc.sync.dma_start(bvl_sb[:], bvl[:])
    nc.sync.dma_start(bo_sb[:], bo_row[:])
    nc.vector.memset(onesf_sb[:], 1.0)

    PS_F = max(C.SQS, 512)  # tag-"s" psum slot free-size (2 banks at 1024)

    stg = {}

    def stage1_x(name, x_in, S):
        # cast-load 256 rows per SWDGE DMA (1 MB): partition p holds rows
        # {st*256+p, st*256+128+p} side by side; the store mirrors the layout
        # so stg stays plain row-major for the xbar reads.
        stg[name] = dram.tile([S, C.DM], BF16, name=f"stg_{name}",
                              uniquify=True)
        for st in range(S // 256):
            t = staging.tile([128, 2 * C.DM], BF16, name="xcast", tag="xcast",
                             bufs=2, padded_shape=[128, 2 * max(C.DM, 1024)])
            tv = t.rearrange("p (a d) -> p a d", a=2)
            nc.gpsimd.dma_start(
                tv,
                x_in[st * 256:(st + 1) * 256, :].rearrange(
                    "(a p) d -> p a d", p=128),
            )
            nc.sync.dma_start(
                stg[name][st * 256:(st + 1) * 256, :].rearrange(
                    "(a p) d -> p a d", p=128),
                tv,
            )

    def load_xT_full(name, S, dst, splits=2):
        """Whole xT tensor: split-column xbar transposes per kt block
        (early splits usable before the tail of staging lands)."""
        Hs = S // splits
        for part in range(splits):
            for kt in range(C.KT):
                nc.sync.dma_start(
                    dst[:, kt * S + part * Hs: kt * S + part * Hs + Hs],
                    stg[name][part * Hs:(part + 1) * Hs, kt * 128:(kt + 1) * 128],
                    transpose=True,
                )

    def finish():
        for pl in reversed(list(pools.values())):
            pl.release()

    # ---------------- v: stage then project ----------------
    # v natural [Sk, d] + ones col: v_sb block j: [128, H*65], head h at
    # cols [65h, 65h+65): cols 65h..65h+63 = v dims, col 65h+64 = ones
    # (so the PV matmul's psum row 64 = Z; v-bias folded into R)
    stage1_x("v", xv, C.SK)
    wv_sb = wv_pool.tile([128, C.KT * C.DM], BF16, name="w_v")
    for kt in range(C.KT):
        nc.scalar.dma_start(wv_sb[:, kt * C.DM:(kt + 1) * C.DM],
                            w_dram["v"][kt * 128:(kt + 1) * 128, :])
    v_view = v_sb.rearrange("p (j h c) -> p j h c", j=C.NJ, c=65)
    xv_sb = xv_pool.tile([128, C.KT * C.SK], BF16, name="xv_sb")
    load_xT_full("v", C.SK, xv_sb)
    for (ns, nw) in slices(C.SK, 512):
        for j in range(ns // 128, (ns + nw) // 128):
            for (ds_, dw) in slices(C.DM, 512):
                hs, hw = ds_ // DK, dw // DK
                ps = ps_s.tile([128, dw], F32, name="ps_v", tag="s",
                               padded_shape=[128, PS_F])
                for kt in range(C.KT):
                    nc.tensor.matmul(
                        ps[:],
                        xv_sb[:, kt * C.SK + j * 128: kt * C.SK + (j + 1) * 128],
                        wv_sb[:, kt * C.DM + ds_: kt * C.DM + ds_ + dw],
                        start=(kt == 0), stop=(kt == C.KT - 1),
                    )
                nc.vector.tensor_copy(
                    v_view[:, j, hs:hs + hw, 0:64],
                    ps.rearrange("p (h c) -> p h c", c=DK),
                )
    nc.vector.memset(v_view[:, :, :, 64:65], 1.0)

    # ---------------- mask + k/q staging (overlaps v-proj) ----------------
    mstg = dram.tile([C.SQ, C.SK], BF16, name="mstg")
    mchunk = min(1024, C.SK)
    for st in range(C.SQ // 128):
        for (cs, cw) in slices(C.SK, mchunk):
            ti = staging.tile([128, cw], I32, name="mint", tag="mint", bufs=2,
                              padded_shape=[128, mchunk])
            nc.gpsimd.dma_start(ti[:], msk[st * 128:(st + 1) * 128, cs:cs + cw])
            tb = staging.tile([128, cw], BF16, name="mbf", tag="mbf", bufs=1,
                              padded_shape=[128, mchunk])
            nc.vector.tensor_copy(tb[:], ti[:])
            nc.scalar.dma_start(mstg[st * 128:(st + 1) * 128, cs:cs + cw], tb[:])
    for j in range(C.NJ):
        nc.sync.dma_start(
            maskT_sb[:, j * C.SQ:(j + 1) * C.SQ],
            mstg[:, j * 128:(j + 1) * 128],
            transpose=True,
        )
    stage1_x("k", xk, C.SK)
    stage1_x("q", xq, C.SQ)

    if C.max_stage <= 2:
        finish()
        return

    # ---------------- k projection (own phase/pool) ----------------
    xv_pool.release()
    del pools["xv"]
    wv_pool.release()
    del pools["wv"]
    staging.release()
    del pools["staging"]
    xk_pool = open_pool("xk", 1)
    xk_sb = xk_pool.tile([128, C.KT * C.SK], BF16, name="xk_sb")
    load_xT_full("k", C.SK, xk_sb)
    wk_pool = open_pool("wk", 1)
    wk_sb = wk_pool.tile([128, C.KT * C.DM], BF16, name="wk_sb")
    for kt in range(C.KT):
        nc.scalar.dma_start(wk_sb[:, kt * C.DM:(kt + 1) * C.DM],
                            w_dram["k"][kt * 128:(kt + 1) * 128, :])
    for hp in range(C.HP):
        for (ns, nw) in slices(C.SK, PS_F):
            ps = ps_s.tile([128, nw], F32, name="ps_kp", tag="s",
                           padded_shape=[128, PS_F])
            for (qs, qw) in slices(nw, 512):
                for kt in range(C.KT):
                    nc.tensor.matmul(
                        ps[:, qs:qs + qw],
                        wk_sb[:, kt * C.DM + hp * 128: kt * C.DM + (hp + 1) * 128],
                        xk_sb[:, kt * C.SK + ns + qs: kt * C.SK + ns + qs + qw],
                        start=(kt == 0), stop=(kt == C.KT - 1),
                    )
            nc.scalar.activation(kT_sb[:, hp * C.SK + ns: hp * C.SK + ns + nw],
                                 ps[:], AF.Identity, bias=bkl_sb[:, hp:hp + 1])
    wk_pool.release()
    del pools["wk"]
    xk_pool.release()
    del pools["xk"]

    # ---------------- q projection (own phase/pool) ----------------
    xq_pool = open_pool("xq", 1)
    xq_sb = xq_pool.tile([128, C.KT * C.SQ], BF16, name="xq_sb")
    load_xT_full("q", C.SQ, xq_sb)
    wq_pool = open_pool("wq", 1)
    wq_sb = wq_pool.tile([128, C.KT * C.DM], BF16, name="wq_sb")
    for kt in range(C.KT):
        nc.scalar.dma_start(wq_sb[:, kt * C.DM:(kt + 1) * C.DM],
                            w_dram["q"][kt * 128:(kt + 1) * 128, :])
    for hp in range(C.HP):
        for (ns, nw) in slices(C.SQ, PS_F):
            ps = ps_s.tile([128, nw], F32, name="ps_qp", tag="s",
                           padded_shape=[128, PS_F])
            for (qs, qw) in slices(nw, 512):
                for kt in range(C.KT):
                    nc.tensor.matmul(
                        ps[:, qs:qs + qw],
                        wq_sb[:, kt * C.DM + hp * 128: kt * C.DM + (hp + 1) * 128],
                        xq_sb[:, kt * C.SQ + ns + qs: kt * C.SQ + ns + qs + qw],
                        start=(kt == 0), stop=(kt == C.KT - 1),
                    )
            nc.scalar.activation(qT_sb[:, hp * C.SQ + ns: hp * C.SQ + ns + nw],
                                 ps[:], AF.Identity, bias=bql_sb[:, hp:hp + 1])
    wq_pool.release()
    del pools["wq"]
    xq_pool.release()
    del pools["xq"]

    wo_pool = open_pool("wo", 1)
    wo_sb = wo_pool.tile([128, C.KT * C.DM], BF16, name="wo_sb")
    for kt in range(C.KT):
        nc.scalar.dma_start(wo_sb[:, kt * C.DM:(kt + 1) * C.DM],
                            w_dram["o"][kt * 128:(kt + 1) * 128, :])
    attn = open_pool("attn", 1)

    for (sq, sw) in slices(C.SQ, C.SQS):
        for hp in range(C.HP):
            pv = [
                ps_pv.tile([65, sw], F32, name=f"ps_pv{i}", tag="pv",
                           padded_shape=[65, PS_F])
                for i in range(2)
            ]
            # software pipeline: scores/exp/mask run PIPE iterations ahead of
            # the PV matmuls so the in-order PE stream never stalls on the
            # ACT(exp) -> DVE(mask) chain of its own iteration.
            PIPE = 3
            pm_hist = []

            def emit_pv(jj, pms, pv=pv, hp=hp):
                for i in range(2):
                    for (qs, qw) in slices(sw, 512):
                        nc.tensor.matmul(
                            pv[i][:, qs:qs + qw], v_view[:, jj, 2 * hp + i, :],
                            pms[i][:, qs:qs + qw],
                            start=(jj == 0), stop=(jj == C.NJ - 1),
                        )

            for j in range(C.NJ):
                pms = []
                sss = [ps_s.tile([128, sw], F32, name=f"ps_sc{i}", tag="s",
                                 padded_shape=[128, PS_F]) for i in range(2)]
                # interleave the two heads' MMs so the row-tiled (0,0)/(64,0)
                # pairs sit adjacent in the PE queue and run concurrently
                for (qs, qw) in slices(sw, 512):
                    for i in range(2):
                        nc.tensor.matmul(
                            sss[i][:, qs:qs + qw],
                            kT_sb[i * 64:(i + 1) * 64,
                                  hp * C.SK + j * 128: hp * C.SK + (j + 1) * 128],
                            qT_sb[i * 64:(i + 1) * 64,
                                  hp * C.SQ + sq + qs: hp * C.SQ + sq + qs + qw],
                            start=True, stop=True,
                        )
                for i in range(2):
                    pe = attn.tile([128, sw], BF16, name="p_exp", tag="pexp",
                                   bufs=3, padded_shape=[128, C.SQS])
                    nc.scalar.activation(pe[:], sss[i][:], AF.Exp)
                    pm = attn.tile([128, sw], BF16, name="p_msk", tag="pmask",
                                   bufs=6, padded_shape=[128, C.SQS])
                    nc.vector.tensor_tensor(
                        out=pm[:], in0=pe[:],
                        in1=maskT_sb[:, j * C.SQ + sq: j * C.SQ + sq + sw],
                        op=ALU.mult,
                    )
                    pms.append(pm)
                pm_hist.append((j, pms))
                if len(pm_hist) > PIPE:
                    jj, pp = pm_hist.pop(0)
                    emit_pv(jj, pp)
            for jj, pp in pm_hist:
                emit_pv(jj, pp)
            for i in range(2):
                # Z row 64 -> sbuf; PE-broadcast; approx-reciprocal; rows 0..63
                zrow = attn.tile([65, sw], F32, name="zrow", tag="zrow", bufs=2,
                                 padded_shape=[65, C.SQS])
                nc.vector.tensor_copy(zrow[64:65, :], pv[i][64:65, :])
                zb = ps_s.tile([64, sw], F32, name="zb", tag="s",
                               padded_shape=[128, PS_F])
                for (qs, qw) in slices(sw, 512):
                    nc.tensor.matmul(zb[:, qs:qs + qw], onesf_sb[64:65, 0:64],
                                     zrow[64:65, qs:qs + qw],
                                     start=True, stop=True)
                zr = attn.tile([64, sw], F32, name="zr", tag="zr", bufs=2,
                               padded_shape=[64, C.SQS])
                nc.vector.reciprocal_approx_fast(out=zr[:], in_=zb[:])
                tmp = attn.tile([64, sw], BF16, name="xat_t", tag="xat_t", bufs=1,
                                padded_shape=[64, C.SQS])
                nc.vector.tensor_tensor(out=tmp[:], in0=pv[i][0:64, :],
                                        in1=zr[:], op=ALU.mult)
                # partition hop: rows 0..63 -> xattnT pair rows 64i..64i+64
                nc.sync.dma_start(
                    xattnT_sb[64 * i:64 * (i + 1), hp * C.SQ + sq: hp * C.SQ + sq + sw],
                    tmp[:],
                )

    if C.max_stage <= 3:
        finish()
        return

    # ---------------- epilogue: R = bv@WoT + bo, then output projection ----
    attn.release()
    del pools["attn"]
    epi = open_pool("epi", 1)

    # R = bv@WoT + bo
    for (ns, nw) in slices(C.DM, 512):
        psR = ps_s.tile([1, nw], F32, name="psR", tag="s", padded_shape=[128, PS_F])
        for kt in range(C.KT):
            nc.tensor.matmul(
                psR[:], bvl_sb[:, kt:kt + 1],
                wo_sb[:, kt * C.DM + ns: kt * C.DM + ns + nw],
                start=(kt == 0), stop=(kt == C.KT - 1),
            )
        nc.vector.tensor_tensor(out=Rrow_sb[0:1, ns:ns + nw], in0=psR[:],
                                in1=bo_sb[0:1, ns:ns + nw], op=ALU.add)
        psB = ps_s.tile([128, nw], F32, name="psB", tag="s", padded_shape=[128, PS_F])
        nc.tensor.matmul(psB[:], onesf_sb[0:1, :], Rrow_sb[0:1, ns:ns + nw],
                         start=True, stop=True)
        nc.vector.tensor_copy(R_sb[:, ns:ns + nw], psB[:])


    for m in range(C.SQ // 128):
        for (ns, nw) in slices(C.DM, PS_F):
            ps = ps_pv.tile([128, nw], F32, name="ps_o", tag="pv",
                            padded_shape=[128, PS_F])
            for (qs, qw) in slices(nw, 512):
                for hp in range(C.HP):
                    nc.tensor.matmul(
                        ps[:, qs:qs + qw],
                        xattnT_sb[:, hp * C.SQ + m * 128: hp * C.SQ + (m + 1) * 128],
                        wo_sb[:, hp * C.DM + ns + qs: hp * C.DM + ns + qs + qw],
                        start=(hp == 0), stop=(hp == C.HP - 1),
                    )
            ot = epi.tile([128, nw], F32, name="out_sb", tag="out_sb", bufs=2,
                          padded_shape=[128, PS_F])
            nc.vector.tensor_tensor(out=ot[:], in0=ps[:], in1=R_sb[:, ns:ns + nw],
                                    op=ALU.add)
            nc.sync.dma_start(out[m * 128:(m + 1) * 128, ns:ns + nw], ot[:])

    finish()


def build(cfg, reps=1):
    nc = bacc.Bacc("TRN2", target_bir_lowering=False, debug=False)
    C = cfg
    io = {
        "xq": nc.dram_tensor("xq", [C.SQ, C.DM], F32, kind="ExternalInput").ap(),
        "xk": nc.dram_tensor("xk", [C.SK, C.DM], F32, kind="ExternalInput").ap(),
        "xv": nc.dram_tensor("xv", [C.SK, C.DM], F32, kind="ExternalInput").ap(),
        "mask": nc.dram_tensor("mask", [C.SQ, C.SK], I32, kind="ExternalInput").ap(),
        "wqt": nc.dram_tensor("wqt", [C.DM, C.DM], BF16, kind="ExternalInput").ap(),
        "wkt": nc.dram_tensor("wkt", [C.DM, C.DM], BF16, kind="ExternalInput").ap(),
        "wvt": nc.dram_tensor("wvt", [C.DM, C.DM], BF16, kind="ExternalInput").ap(),
        "wot": nc.dram_tensor("wot", [C.DM, C.DM], BF16, kind="ExternalInput").ap(),
        "bql": nc.dram_tensor("bql", [128, C.HP], F32, kind="ExternalInput").ap(),
        "bkl": nc.dram_tensor("bkl", [128, C.HP], F32, kind="ExternalInput").ap(),
        "bvl": nc.dram_tensor("bvl", [128, C.KT], BF16, kind="ExternalInput").ap(),
        "bo_row": nc.dram_tensor("bo_row", [1, C.DM], F32, kind="ExternalInput").ap(),
        "out": nc.dram_tensor("out", [C.SQ, C.DM], F32, kind="ExternalOutput").ap(),
    }
    with tile.TileContext(nc) as tc:
        for _ in range(reps):
            emit_kernel(tc, cfg, io)
    nc.compile()
    return nc


def host_prep(query, key, value, mask, Wq, bq, Wk, bk, Wv, bv, Wo, bo, cfg):
    """Host-side layout prep (weight transpose/cast, per-core slicing)."""
    C = cfg
    bf = ml_dtypes.bfloat16
    wqt = np.ascontiguousarray((Wq.T * 0.125).astype(bf))   # 1/sqrt(dk) folded
    wkt = np.ascontiguousarray(Wk.T.astype(bf))
    wvt = np.ascontiguousarray(Wv.T.astype(bf))
    wot = np.ascontiguousarray(Wo.T.astype(bf))
    bql = np.ascontiguousarray((bq * 0.125).reshape(C.HP, 128).T.astype(np.float32))
    bkl = np.ascontiguousarray(bk.reshape(C.HP, 128).T.astype(np.float32))
    bvl = np.ascontiguousarray(bv.reshape(C.KT, 128).T.astype(bf))
    bo_row = np.ascontiguousarray(bo.reshape(1, C.DM).astype(np.float32))
    shared = dict(wqt=wqt, wkt=wkt, wvt=wvt, wot=wot, bql=bql, bkl=bkl,
                  bvl=bvl, bo_row=bo_row)
    in_maps = []
    B = query.shape[0]
    halves = query.shape[1] // C.SQ
    for c in range(B * halves):
        b, h = divmod(c, halves)
        m = dict(shared)
        m["xq"] = np.ascontiguousarray(query[b, h * C.SQ:(h + 1) * C.SQ, :])
        m["xk"] = np.ascontiguousarray(key[b])
        m["xv"] = np.ascontiguousarray(value[b])
        m["mask"] = np.ascontiguousarray(mask[b, h * C.SQ:(h + 1) * C.SQ, :])
        in_maps.append(m)
    return in_maps


_CACHED = {}


def get_built():
    if "nc" not in _CACHED:
        _CACHED["nc"] = build(Cfg())
    return _CACHED["nc"]


def kernel(query, key, value, mask, Wq, bq, Wk, bk, Wv, bv, Wo, bo):
    cfg = Cfg()
    nc = get_built()
    in_maps = host_prep(query, key, value, mask, Wq, bq, Wk, bk, Wv, bv, Wo, bo, cfg)
    res = run_bass_kernel_spmd(nc, in_maps, core_ids=list(range(N_CORES)))
    B, S, DM = query.shape
    out = np.empty((B, S, DM), np.float32)
    for c in range(N_CORES):
        b, h = divmod(c, 2)
        out[b, h * cfg.SQ:(h + 1) * cfg.SQ, :] = res.results[c]["out"]
    return out



# revision 44
# speedup vs baseline: 1.5284x; 1.5284x over previous
"""MultiHeadedAttention Trainium2 kernel (8-core SPMD, data-parallel).

Sharding: 8 cores = (batch b in 0..3) x (query half in 0..1). Each core
computes out[b, half*1024:(half+1)*1024, :] independently - no collectives.

v2 pipeline (vs v1: no DRAM staging, per-head-pair proj/attention overlap):
  - host casts x/mask to bf16; device DMA-transposes them straight from the
    input DRAM tensors into SBUF (xbar transpose, 2-byte dtype)
  - V projection runs first (PE-dense) while k/q/mask transposes stream in;
    mask converts bf16->fp8 in SBUF (exact for 0/1) to halve its footprint
  - per head pair hp: K/Q projection for hp+1 is emitted inside the
    attention j-loop of hp, so PE never drains while ACT (exp) is busy
  - scoresT [k,q] psum -> exp on ScalarE -> mask multiply on the otherwise
    idle Pool engine -> PV accumulate (Z via trailing ones column of v)
  - finalize: PE-broadcast Z, reciprocal_approx_fast, multiply, DMA hop
    into xattnT [dm, q]; epilogue out = xattnT.T @ WoT + R, R = bv@WoT + bo
"""
import numpy as np
import ml_dtypes

import concourse.bass as bass
import concourse.mybir as mybir
import concourse.tile as tile
from concourse import bacc
from concourse.bass_utils import run_bass_kernel_spmd

F32 = mybir.dt.float32
BF16 = mybir.dt.bfloat16
FP8 = mybir.dt.float8e4
AF = mybir.ActivationFunctionType
ALU = mybir.AluOpType

N_CORES = 8
DK = 64


def slices(total, chunk):
    return [(s, min(chunk, total - s)) for s in range(0, total, chunk)]


class Cfg:
    def __init__(self, SQ=1024, SK=2048, DM=1024, H=16, max_stage=5):
        assert DM % 128 == 0 and SK % 128 == 0 and SQ % 128 == 0 and H % 2 == 0
        self.SQ, self.SK, self.DM, self.H = SQ, SK, DM, H
        self.KT = DM // 128          # dm contraction chunks
        self.HP = H // 2             # head pairs
        self.NJ = SK // 128          # Sk tiles
        self.SQS = min(1024, SQ)     # attention Sq slice width (2 psum banks)
        self.max_stage = max_stage
        assert SQ % self.SQS == 0
        assert H * DK == DM


def emit_kernel(tc, cfg, io):
    nc = tc.nc
    C = cfg
    xq, xk, xv, msk = io["xq"], io["xk"], io["xv"], io["mask"]
    w_dram = {"q": io["wqt"], "k": io["wkt"], "v": io["wvt"], "o": io["wot"]}
    bql, bkl, bvl, bo_row = io["bql"], io["bkl"], io["bvl"], io["bo_row"]
    out = io["out"]
    PS_F = max(C.SQS, 512)

    pools = {}

    def open_pool(name, bufs=1, space="SBUF"):
        pools[name] = tc.alloc_tile_pool(name=name, bufs=bufs, space=space)
        return pools[name]

    persist = open_pool("persist", 1)
    rot = open_pool("rot", 1)
    ps_s = open_pool("ps_s", 2, space="PSUM")
    ps_pv = open_pool("ps_pv", 2, space="PSUM")
    work = open_pool("work", 1)
    poolA = open_pool("A", 1)   # LIFO: A on top so it can release mid-kernel

    # ---------------- persistent tiles ----------------
    # per-DMA-chunk tiles: a multi-DMA fill of ONE tile serializes on Tile's
    # same-tile WAW semaphores (~5us per DMA round trip), so every DMA gets
    # its own tile
    v_sb = persist.tile([128, C.NJ * C.H * 65], BF16, name="v_sb")
    maskT_t = [persist.tile([128, C.SQ], BF16, name=f"maskT{j}")
               for j in range(C.NJ)]
    xkT_t = [persist.tile([128, C.SK], BF16, name=f"xkT{kt}")
             for kt in range(C.KT)]
    xqT_t = [persist.tile([128, C.SQ], BF16, name=f"xqT{kt}")
             for kt in range(C.KT)]
    R_sb = persist.tile([128, C.DM], BF16, name="R_sb")
    bql_sb = persist.tile([128, C.HP], F32, name="bql_sb")
    bkl_sb = persist.tile([128, C.HP], F32, name="bkl_sb")
    bvl_sb = persist.tile([128, C.KT], BF16, name="bvl_sb")
    bo_sb = persist.tile([1, C.DM], BF16, name="bo_sb")
    onesb = persist.tile([1, 128], BF16, name="onesb")

    xvT_t = [poolA.tile([128, C.SK], BF16, name=f"xvT{kt}")
             for kt in range(C.KT)]
    wv_t = [poolA.tile([128, C.DM], BF16, name=f"wv{kt}") for kt in range(C.KT)]

    v_view = v_sb.rearrange("p (j h c) -> p j h c", j=C.NJ, c=65)

    # rotating double-buffered tiles, memoized so every use of (kind, hp)
    # shares one handle (a second pool.tile() call would alias a NEW tile
    # into the slot instead of reading what the projection wrote)
    _rot_tiles = {}

    def _rot(kind, hp, shape, nbuf=2):
        key = (kind, hp)
        if key not in _rot_tiles:
            _rot_tiles[key] = rot.tile(shape, BF16, name=f"{kind}{hp % nbuf}",
                                       tag=f"{kind}{hp % nbuf}")
        return _rot_tiles[key]

    def kT_buf(hp):
        return _rot("kT", hp, [128, C.SK])

    def qT_buf(hp):
        return _rot("qT", hp, [128, C.SQ])

    # single-buffered: wkh(hp) is fully consumed by proj(hp) during
    # attention(hp-1), before wkh(hp+1)'s load is issued
    def wkh_buf(hp):
        return _rot("wk", hp, [128, C.KT * 128], nbuf=1)

    def wqh_buf(hp):
        return _rot("wq", hp, [128, C.KT * 128], nbuf=1)

    # ---------------- prologue DMAs ----------------
    nc.gpsimd.dma_start(bql_sb[:], bql[:])
    nc.gpsimd.dma_start(bkl_sb[:], bkl[:])
    nc.gpsimd.dma_start(bvl_sb[:], bvl[:])
    nc.gpsimd.dma_start(bo_sb[:], bo_row[:])
    nc.vector.memset(onesb[:], 1.0)
    nc.vector.memset(v_view[:, :, :, 64:65], 1.0)

    # DMA phase discipline: copies and transposes must not coexist in the
    # schedule window (Tile serializes every DMACopy<->DmaTranspose pair,
    # ~4us each, to dodge a real HW xbar deadlock). All prologue copies
    # first, then all transposes (SP queue only - dual-queue transposes
    # corrupt data on HW).
    def load_wh(dst, name_w, hp):
        # [1024, 128] column slice -> [128, KT*128] (kt-blocked), one DMA
        nc.sync.dma_start(
            dst.rearrange("p (k c) -> p k c", k=C.KT),
            w_dram[name_w][:, hp * 128:(hp + 1) * 128].rearrange(
                "(k p) c -> p k c", p=128),
        )

    for kt in range(C.KT):
        nc.gpsimd.dma_start(wv_t[kt][:], w_dram["v"][kt * 128:(kt + 1) * 128, :])
    load_wh(wkh_buf(0), "k", 0)
    load_wh(wqh_buf(0), "q", 0)

    for kt in range(C.KT):
        nc.sync.dma_start(
            xvT_t[kt][:], xv[:, kt * 128:(kt + 1) * 128], transpose=True)
    for kt in range(C.KT):
        nc.sync.dma_start(
            xkT_t[kt][:], xk[:, kt * 128:(kt + 1) * 128], transpose=True)
    for kt in range(C.KT):
        nc.sync.dma_start(
            xqT_t[kt][:], xq[:, kt * 128:(kt + 1) * 128], transpose=True)

    # ---------------- V projection (PE-dense pipeline fill) ----------------
    def v_proj_j(j):
        ps = ps_s.tile([128, C.DM], F32, name="ps_v", tag="s",
                       padded_shape=[128, PS_F])
        for (ds_, dw) in slices(C.DM, 512):
            for kt in range(C.KT):
                nc.tensor.matmul(
                    ps[:, ds_:ds_ + dw],
                    xvT_t[kt][:, j * 128:(j + 1) * 128],
                    wv_t[kt][:, ds_:ds_ + dw],
                    start=(kt == 0), stop=(kt == C.KT - 1),
                )
        nc.vector.tensor_copy(
            v_view[:, j, :, 0:64],
            ps.rearrange("p (h c) -> p h c", c=DK),
        )

    for j in range(C.NJ):
        v_proj_j(j)

    # mask: bf16 transposes from DRAM, one per-j tile each (no WAW chain)
    for j in range(C.NJ):
        nc.sync.dma_start(maskT_t[j][:], msk[:, j * 128:(j + 1) * 128],
                          transpose=True)

    poolA.release()
    del pools["A"]
    poolB = open_pool("B", 1)
    xattnT = poolB.tile([128, C.HP * C.SQ], BF16, name="xattnT")
    wo_t = [poolB.tile([128, C.DM], BF16, name=f"wo{kt}") for kt in range(C.KT)]
    for kt in range(C.KT):
        nc.gpsimd.dma_start(wo_t[kt][:], w_dram["o"][kt * 128:(kt + 1) * 128, :])

    if C.max_stage <= 2:
        for pl in reversed(list(pools.values())):
            pl.release()
        return

    # ---------------- projections (emitted per head pair) ----------------
    # projection psums are 512 wide: short "s"-slot holds so the scores/exp
    # rotation stalls at most ~1.7us when a proj tile steals a slot
    def _proj_chunk(xT_t, w, dst, bias_col, ns, nw):
        ps = ps_s.tile([128, nw], F32, name="ps_kp", tag="s",
                       padded_shape=[128, PS_F])
        for kt in range(C.KT):
            nc.tensor.matmul(
                ps[:],
                w[:, kt * 128:(kt + 1) * 128],
                xT_t[kt][:, ns: ns + nw],
                start=(kt == 0), stop=(kt == C.KT - 1),
            )
        nc.vector.tensor_scalar_add(out=dst[:, ns:ns + nw], in0=ps[:],
                                    scalar1=bias_col)

    def proj_chunks(hp, kT, wk, qT, wq):
        """Closures emitting one 512-wide projection chunk each (4 K + 2 Q)."""
        out = []
        for (ns, nw) in slices(C.SK, 512):
            out.append(lambda ns=ns, nw=nw: _proj_chunk(
                xkT_t, wk, kT, bkl_sb[:, hp:hp + 1], ns, nw))
        for (ns, nw) in slices(C.SQ, 512):
            out.append(lambda ns=ns, nw=nw: _proj_chunk(
                xqT_t, wq, qT, bql_sb[:, hp:hp + 1], ns, nw))
        return out

    def k_proj(hp, kT, wk):
        for (ns, nw) in slices(C.SK, 512):
            _proj_chunk(xkT_t, wk, kT, bkl_sb[:, hp:hp + 1], ns, nw)

    def q_proj(hp, qT, wq):
        for (ns, nw) in slices(C.SQ, 512):
            _proj_chunk(xqT_t, wq, qT, bql_sb[:, hp:hp + 1], ns, nw)

    def emit_R():
        # R = bv@WoT + bo, PE-broadcast to 128 rows
        psR = ps_s.tile([1, C.DM], F32, name="psR", tag="s",
                        padded_shape=[128, PS_F])
        for (ns, nw) in slices(C.DM, 512):
            for kt in range(C.KT):
                nc.tensor.matmul(
                    psR[0:1, ns:ns + nw], bvl_sb[:, kt:kt + 1],
                    wo_t[kt][:, ns:ns + nw],
                    start=(kt == 0), stop=(kt == C.KT - 1),
                )
        Rrow = work.tile([1, C.DM], BF16, name="Rrow", tag="zrow", bufs=1,
                         padded_shape=[1, max(C.DM, C.SQS)])
        nc.vector.tensor_tensor(out=Rrow[:], in0=psR[:], in1=bo_sb[:],
                                op=ALU.add)
        psB = ps_s.tile([128, C.DM], F32, name="psB", tag="s",
                        padded_shape=[128, PS_F])
        for (ns, nw) in slices(C.DM, 512):
            nc.tensor.matmul(psB[:, ns:ns + nw], onesb[0:1, :],
                             Rrow[0:1, ns:ns + nw], start=True, stop=True)
        nc.vector.tensor_copy(R_sb[:], psB[:])

    k_proj(0, kT_buf(0), wkh_buf(0))
    q_proj(0, qT_buf(0), wqh_buf(0))

    # ---------------- attention, one head pair at a time ----------------
    for hp in range(C.HP):
        kT = kT_buf(hp)
        qT = qT_buf(hp)
        if hp + 1 < C.HP:
            load_wh(wkh_buf(hp + 1), "k", hp + 1)
            load_wh(wqh_buf(hp + 1), "q", hp + 1)
            nxt_proj = proj_chunks(hp + 1, kT_buf(hp + 1), wkh_buf(hp + 1),
                                   qT_buf(hp + 1), wqh_buf(hp + 1))
        else:
            nxt_proj = []
        for (sq, sw) in slices(C.SQ, C.SQS):
            pv = [
                ps_pv.tile([65, sw], F32, name=f"ps_pv{i}", tag="pv",
                           padded_shape=[65, PS_F])
                for i in range(2)
            ]
            PIPE = 2
            pm_hist = []

            def emit_head(j, i, hp=hp, kT=kT, qT=qT, sq=sq, sw=sw):
                """Scores MMs + exp + mask for one (j, head)."""
                ss = ps_s.tile([128, sw], F32, name=f"ps_sc{i}", tag="s",
                               padded_shape=[128, PS_F])
                for (qs, qw) in slices(sw, 512):
                    nc.tensor.matmul(
                        ss[:, qs:qs + qw],
                        kT[i * 64:(i + 1) * 64, j * 128:(j + 1) * 128],
                        qT[i * 64:(i + 1) * 64, sq + qs: sq + qs + qw],
                        start=True, stop=True,
                    )
                pe = work.tile([128, sw], BF16, name="p_exp", tag="pe",
                               bufs=2, padded_shape=[128, C.SQS])
                nc.scalar.activation(pe[:], ss[:], AF.Exp)
                pm = work.tile([128, sw], BF16, name="p_msk", tag="pm",
                               bufs=4, padded_shape=[128, C.SQS])
                nc.vector.tensor_tensor(
                    out=pm[:], in0=pe[:],
                    in1=maskT_t[j][:, sq: sq + sw],
                    op=ALU.mult,
                )
                return pm

            def emit_pv_half(jj, pmi, i, pv=pv, hp=hp, sw=sw):
                for (qs, qw) in slices(sw, 512):
                    nc.tensor.matmul(
                        pv[i][:, qs:qs + qw], v_view[:, jj, 2 * hp + i, :],
                        pmi[:, qs:qs + qw],
                        start=(jj == 0), stop=(jj == C.NJ - 1),
                    )

            def finalize_head(i, pv=pv, hp=hp, sq=sq, sw=sw):
                # copy PV rows out of PSUM immediately (DVE) and the Z row on
                # ACT, so the pv accumulator slot frees for the next head
                # pair ~3us sooner; normalize from the SBUF copies
                pvc = work.tile([64, sw], BF16, name="pvc", tag="pvc",
                                bufs=2, padded_shape=[64, C.SQS])
                nc.vector.tensor_copy(pvc[:], pv[i][0:64, :])
                zrow = work.tile([1, sw], BF16, name="zrow", tag="zrow",
                                 bufs=1, padded_shape=[1, max(C.DM, C.SQS)])
                nc.scalar.copy(zrow[0:1, :], pv[i][64:65, :])
                zb = ps_s.tile([64, sw], F32, name="zb", tag="s",
                               padded_shape=[128, PS_F])
                for (qs, qw) in slices(sw, 512):
                    nc.tensor.matmul(zb[:, qs:qs + qw], onesb[0:1, 0:64],
                                     zrow[0:1, qs:qs + qw],
                                     start=True, stop=True)
                zr = work.tile([64, sw], F32, name="zr", tag="zr", bufs=1,
                               padded_shape=[64, C.SQS])
                nc.vector.reciprocal_approx_fast(out=zr[:], in_=zb[:])
                tmp = work.tile([64, sw], BF16, name="xat_t", tag="xat_t",
                                bufs=1, padded_shape=[64, C.SQS])
                nc.vector.tensor_tensor(out=tmp[:], in0=pvc[:],
                                        in1=zr[:], op=ALU.mult)
                nc.sync.dma_start(
                    xattnT[64 * i:64 * (i + 1), hp * C.SQ + sq: hp * C.SQ + sq + sw],
                    tmp[:],
                )

            # software-pipelined emission, PE queue order per iteration:
            # [PV halves (deps long ready), proj chunk, scores j+1] so the
            # head-of-queue never stalls on exp's psum-slot release
            pm_hist.append((0, [emit_head(0, 0), emit_head(0, 1)]))
            for j in range(C.NJ):
                if len(pm_hist) >= PIPE:
                    jj, pp = pm_hist.pop(0)
                    emit_pv_half(jj, pp[0], 0)
                    emit_pv_half(jj, pp[1], 1)
                # overlap next head pair's projections with this attention,
                # one 512-chunk per j step so slot steals stay short
                if j >= 4 and j % 2 == 0 and nxt_proj:
                    nxt_proj.pop(0)()
                if j == 3 and hp == 1:
                    emit_R()
                if j + 1 < C.NJ:
                    pms_n = [emit_head(j + 1, 0), emit_head(j + 1, 1)]
                    pm_hist.append((j + 1, pms_n))
            # drain: finalize each head right after its last PV half so
            # head0's normalization overlaps head1's tail
            for idx, (jj, pp) in enumerate(pm_hist):
                last = idx == len(pm_hist) - 1
                emit_pv_half(jj, pp[0], 0)
                if last:
                    finalize_head(0)
                emit_pv_half(jj, pp[1], 1)
                if last:
                    finalize_head(1)

    if C.max_stage <= 3:
        for pl in reversed(list(pools.values())):
            pl.release()
        return

    # ---------------- epilogue: output projection ----------------
    for m in range(C.SQ // 128):
        ps = ps_pv.tile([128, C.DM], F32, name="ps_o", tag="pv",
                        padded_shape=[128, PS_F])
        for (qs, qw) in slices(C.DM, 512):
            for hp in range(C.HP):
                nc.tensor.matmul(
                    ps[:, qs:qs + qw],
                    xattnT[:, hp * C.SQ + m * 128: hp * C.SQ + (m + 1) * 128],
                    wo_t[hp][:, qs:qs + qw],
                    start=(hp == 0), stop=(hp == C.HP - 1),
                )
        ot = work.tile([128, C.DM], BF16, name="out_sb", tag="out_sb", bufs=1,
                       padded_shape=[128, PS_F])
        nc.vector.tensor_tensor(out=ot[:], in0=ps[:], in1=R_sb[:], op=ALU.add)
        # SWDGE cast-DMA bf16 -> f32 (Pool is idle in the epilogue)
        nc.gpsimd.dma_start(out[m * 128:(m + 1) * 128, :], ot[:])

    for pl in reversed(list(pools.values())):
        pl.release()


def build(cfg, reps=1):
    nc = bacc.Bacc("TRN2", target_bir_lowering=False, debug=False)
    C = cfg
    io = {
        "xq": nc.dram_tensor("xq", [C.SQ, C.DM], BF16, kind="ExternalInput").ap(),
        "xk": nc.dram_tensor("xk", [C.SK, C.DM], BF16, kind="ExternalInput").ap(),
        "xv": nc.dram_tensor("xv", [C.SK, C.DM], BF16, kind="ExternalInput").ap(),
        "mask": nc.dram_tensor("mask", [C.SQ, C.SK], BF16, kind="ExternalInput").ap(),
        "wqt": nc.dram_tensor("wqt", [C.DM, C.DM], BF16, kind="ExternalInput").ap(),
        "wkt": nc.dram_tensor("wkt", [C.DM, C.DM], BF16, kind="ExternalInput").ap(),
        "wvt": nc.dram_tensor("wvt", [C.DM, C.DM], BF16, kind="ExternalInput").ap(),
        "wot": nc.dram_tensor("wot", [C.DM, C.DM], BF16, kind="ExternalInput").ap(),
        "bql": nc.dram_tensor("bql", [128, C.HP], F32, kind="ExternalInput").ap(),
        "bkl": nc.dram_tensor("bkl", [128, C.HP], F32, kind="ExternalInput").ap(),
        "bvl": nc.dram_tensor("bvl", [128, C.KT], BF16, kind="ExternalInput").ap(),
        "bo_row": nc.dram_tensor("bo_row", [1, C.DM], BF16, kind="ExternalInput").ap(),
        "out": nc.dram_tensor("out", [C.SQ, C.DM], F32, kind="ExternalOutput").ap(),
    }
    with tile.TileContext(nc) as tc:
        for _ in range(reps):
            emit_kernel(tc, cfg, io)
    nc.compile()
    return nc


def host_prep(query, key, value, mask, Wq, bq, Wk, bk, Wv, bv, Wo, bo, cfg):
    """Host-side layout prep (weight transpose/cast, bf16 casts, slicing)."""
    C = cfg
    bf = ml_dtypes.bfloat16
    wqt = np.ascontiguousarray((Wq.T * 0.125).astype(bf))   # 1/sqrt(dk) folded
    wkt = np.ascontiguousarray(Wk.T.astype(bf))
    wvt = np.ascontiguousarray(Wv.T.astype(bf))
    wot = np.ascontiguousarray(Wo.T.astype(bf))
    bql = np.ascontiguousarray((bq * 0.125).reshape(C.HP, 128).T.astype(np.float32))
    bkl = np.ascontiguousarray(bk.reshape(C.HP, 128).T.astype(np.float32))
    bvl = np.ascontiguousarray(bv.reshape(C.KT, 128).T.astype(bf))
    bo_row = np.ascontiguousarray(bo.reshape(1, C.DM).astype(bf))
    shared = dict(wqt=wqt, wkt=wkt, wvt=wvt, wot=wot, bql=bql, bkl=bkl,
                  bvl=bvl, bo_row=bo_row)
    in_maps = []
    B = query.shape[0]
    halves = query.shape[1] // C.SQ
    key_bf = [np.ascontiguousarray(key[b].astype(bf)) for b in range(B)]
    val_bf = [np.ascontiguousarray(value[b].astype(bf)) for b in range(B)]
    for c in range(B * halves):
        b, h = divmod(c, halves)
        m = dict(shared)
        m["xq"] = np.ascontiguousarray(
            query[b, h * C.SQ:(h + 1) * C.SQ, :].astype(bf))
        m["xk"] = key_bf[b]
        m["xv"] = val_bf[b]
        m["mask"] = np.ascontiguousarray(
            mask[b, h * C.SQ:(h + 1) * C.SQ, :].astype(bf))
        in_maps.append(m)
    return in_maps


_CACHED = {}


def get_built():
    if "nc" not in _CACHED:
        _CACHED["nc"] = build(Cfg())
    return _CACHED["nc"]


def kernel(query, key, value, mask, Wq, bq, Wk, bk, Wv, bv, Wo, bo):
    cfg = Cfg()
    nc = get_built()
    in_maps = host_prep(query, key, value, mask, Wq, bq, Wk, bk, Wv, bv, Wo, bo, cfg)
    res = run_bass_kernel_spmd(nc, in_maps, core_ids=list(range(N_CORES)))
    B, S, DM = query.shape
    out = np.empty((B, S, DM), np.float32)
    for c in range(N_CORES):
        b, h = divmod(c, 2)
        out[b, h * cfg.SQ:(h + 1) * cfg.SQ, :] = res.results[c]["out"]
    return out


# revision 51
# speedup vs baseline: 1.9901x; 1.3021x over previous
"""MultiHeadedAttention Trainium2 kernel (8-core SPMD, data-parallel).

Sharding: 8 cores = (batch b in 0..3) x (query half in 0..1). Each core
computes out[b, half*1024:(half+1)*1024, :] independently - no collectives.

v2 pipeline (vs v1: no DRAM staging, per-head-pair proj/attention overlap):
  - host casts x/mask to bf16; device DMA-transposes them straight from the
    input DRAM tensors into SBUF (xbar transpose, 2-byte dtype). All
    transposes on the SP queue only (dual-queue transposes corrupt on HW)
    and phase-separated from DMA copies (Tile serializes copy<->transpose
    pairs ~4us each for the xbar hazard). Every DMA gets its own tile
    (multi-DMA fills of one tile serialize on WAW semaphores).
  - V projection runs first (PE-dense) while k/q/mask transposes stream in
  - per head pair hp: K/Q projection chunks for hp+1 are emitted inside the
    attention j-loop of hp (512-wide psums = short "s"-slot steals), so PE
    never drains while ACT (exp) is busy
  - attention j-loop is software-pipelined: iteration j emits PV halves of
    j-1 (deps long ready), a proj chunk, then scores/exp/mask for j+1 so
    the in-order PE queue never stalls on exp's psum-slot release
  - scoresT [k,q] psum -> exp on ScalarE -> mask multiply on DVE (bf16 2x
    mode) -> PV accumulate (Z via trailing ones column of v)
  - finalize: Z-row copy on ACT, PE-broadcast Z, reciprocal_approx_fast,
    multiply, DMA hop into xattnT [dm, q]
  - epilogue out = xattnT.T @ WoT + R, R = bv@WoT + bo, bf16 out staging
    cast to f32 by SWDGE during the store
"""
import numpy as np
import ml_dtypes

import concourse.bass as bass
import concourse.mybir as mybir
import concourse.tile as tile
from concourse import bacc
from concourse.bass_utils import run_bass_kernel_spmd

F32 = mybir.dt.float32
BF16 = mybir.dt.bfloat16
FP8 = mybir.dt.float8e4
AF = mybir.ActivationFunctionType
ALU = mybir.AluOpType

N_CORES = 8
DK = 64


def slices(total, chunk):
    return [(s, min(chunk, total - s)) for s in range(0, total, chunk)]


class Cfg:
    def __init__(self, SQ=1024, SK=2048, DM=1024, H=16, max_stage=5):
        assert DM % 128 == 0 and SK % 128 == 0 and SQ % 128 == 0 and H % 2 == 0
        self.SQ, self.SK, self.DM, self.H = SQ, SK, DM, H
        self.KT = DM // 128          # dm contraction chunks
        self.HP = H // 2             # head pairs
        self.NJ = SK // 128          # Sk tiles
        self.SQS = min(1024, SQ)     # attention Sq slice width (2 psum banks)
        self.max_stage = max_stage
        assert SQ % self.SQS == 0
        assert H * DK == DM


def emit_kernel(tc, cfg, io):
    nc = tc.nc
    C = cfg
    xq, xk, xv, msk = io["xq"], io["xk"], io["xv"], io["mask"]
    w_dram = {"q": io["wqt"], "k": io["wkt"], "v": io["wvt"], "o": io["wot"]}
    bql, bkl, bvl, bo_row = io["bql"], io["bkl"], io["bvl"], io["bo_row"]
    out = io["out"]
    PS_F = max(C.SQS, 512)

    pools = {}

    def open_pool(name, bufs=1, space="SBUF"):
        pools[name] = tc.alloc_tile_pool(name=name, bufs=bufs, space=space)
        return pools[name]

    persist = open_pool("persist", 1)
    rot = open_pool("rot", 1)
    ps_s = open_pool("ps_s", 2, space="PSUM")
    ps_pv = open_pool("ps_pv", 2, space="PSUM")
    work = open_pool("work", 1)
    poolA = open_pool("A", 1)   # LIFO: A on top so it can release mid-kernel

    # ---------------- persistent tiles ----------------
    # per-DMA-chunk tiles: a multi-DMA fill of ONE tile serializes on Tile's
    # same-tile WAW semaphores (~5us per DMA round trip), so every DMA gets
    # its own tile
    v_sb = persist.tile([128, C.NJ * C.H * 65], BF16, name="v_sb")
    maskT_t = [persist.tile([128, C.SQ], BF16, name=f"maskT{j}")
               for j in range(C.NJ)]
    xkT_t = [persist.tile([128, C.SK], BF16, name=f"xkT{kt}")
             for kt in range(C.KT)]
    xqT_t = [persist.tile([128, C.SQ], BF16, name=f"xqT{kt}")
             for kt in range(C.KT)]
    R_sb = persist.tile([128, C.DM], BF16, name="R_sb")
    bql_sb = persist.tile([128, C.HP], F32, name="bql_sb")
    bkl_sb = persist.tile([128, C.HP], F32, name="bkl_sb")
    bvl_sb = persist.tile([128, C.KT], BF16, name="bvl_sb")
    bo_sb = persist.tile([1, C.DM], BF16, name="bo_sb")
    onesb = persist.tile([1, 128], BF16, name="onesb")
    onesf = persist.tile([1, 128], F32, name="onesf")

    xvT_t = [poolA.tile([128, C.SK], BF16, name=f"xvT{kt}")
             for kt in range(C.KT)]
    wv_t = [poolA.tile([128, C.DM], BF16, name=f"wv{kt}") for kt in range(C.KT)]

    v_view = v_sb.rearrange("p (j h c) -> p j h c", j=C.NJ, c=65)

    # rotating double-buffered tiles, memoized so every use of (kind, hp)
    # shares one handle (a second pool.tile() call would alias a NEW tile
    # into the slot instead of reading what the projection wrote)
    _rot_tiles = {}

    def _rot(kind, hp, shape, nbuf=2):
        key = (kind, hp)
        if key not in _rot_tiles:
            _rot_tiles[key] = rot.tile(shape, BF16, name=f"{kind}{hp % nbuf}",
                                       tag=f"{kind}{hp % nbuf}")
        return _rot_tiles[key]

    def kT_buf(hp):
        return _rot("kT", hp, [128, C.SK])

    def qT_buf(hp):
        return _rot("qT", hp, [128, C.SQ])

    # single-buffered: wkh(hp) is fully consumed by proj(hp) during
    # attention(hp-1), before wkh(hp+1)'s load is issued
    def wkh_buf(hp):
        return _rot("wk", hp, [128, C.KT * 128], nbuf=1)

    def wqh_buf(hp):
        return _rot("wq", hp, [128, C.KT * 128], nbuf=1)

    # ---------------- prologue DMAs ----------------
    nc.gpsimd.dma_start(bql_sb[:], bql[:])
    nc.gpsimd.dma_start(bkl_sb[:], bkl[:])
    nc.gpsimd.dma_start(bvl_sb[:], bvl[:])
    nc.gpsimd.dma_start(bo_sb[:], bo_row[:])
    nc.vector.memset(onesb[:], 1.0)
    nc.vector.memset(onesf[:], 1.0)
    nc.vector.memset(v_view[:, :, :, 64:65], 1.0)

    # DMA phase discipline: copies and transposes must not coexist in the
    # schedule window (Tile serializes every DMACopy<->DmaTranspose pair,
    # ~4us each, to dodge a real HW xbar deadlock). All prologue copies
    # first, then all transposes (SP queue only - dual-queue transposes
    # corrupt data on HW).
    def load_wh(dst, name_w, hp):
        # [1024, 128] column slice -> [128, KT*128] (kt-blocked), one DMA
        nc.sync.dma_start(
            dst.rearrange("p (k c) -> p k c", k=C.KT),
            w_dram[name_w][:, hp * 128:(hp + 1) * 128].rearrange(
                "(k p) c -> p k c", p=128),
        )

    for kt in range(C.KT):
        nc.gpsimd.dma_start(wv_t[kt][:], w_dram["v"][kt * 128:(kt + 1) * 128, :])
    load_wh(wkh_buf(0), "k", 0)
    load_wh(wqh_buf(0), "q", 0)

    for kt in range(C.KT):
        nc.sync.dma_start(
            xvT_t[kt][:], xv[:, kt * 128:(kt + 1) * 128], transpose=True)
    for kt in range(C.KT):
        nc.sync.dma_start(
            xkT_t[kt][:], xk[:, kt * 128:(kt + 1) * 128], transpose=True)
    for kt in range(C.KT):
        nc.sync.dma_start(
            xqT_t[kt][:], xq[:, kt * 128:(kt + 1) * 128], transpose=True)

    # ---------------- V projection (PE-dense pipeline fill) ----------------
    def v_proj_j(j):
        ps = ps_s.tile([128, C.DM], F32, name="ps_v", tag="s",
                       padded_shape=[128, PS_F])
        for (ds_, dw) in slices(C.DM, 512):
            for kt in range(C.KT):
                nc.tensor.matmul(
                    ps[:, ds_:ds_ + dw],
                    xvT_t[kt][:, j * 128:(j + 1) * 128],
                    wv_t[kt][:, ds_:ds_ + dw],
                    start=(kt == 0), stop=(kt == C.KT - 1),
                )
        nc.vector.tensor_copy(
            v_view[:, j, :, 0:64],
            ps.rearrange("p (h c) -> p h c", c=DK),
        )

    for j in range(C.NJ):
        v_proj_j(j)

    # mask: bf16 transposes from DRAM, one per-j tile each (no WAW chain)
    for j in range(C.NJ):
        nc.sync.dma_start(maskT_t[j][:], msk[:, j * 128:(j + 1) * 128],
                          transpose=True)

    poolA.release()
    del pools["A"]
    poolB = open_pool("B", 1)
    xattnT = poolB.tile([128, C.HP * C.SQ], BF16, name="xattnT")
    wo_t = [poolB.tile([128, C.DM], BF16, name=f"wo{kt}") for kt in range(C.KT)]
    for kt in range(C.KT):
        nc.gpsimd.dma_start(wo_t[kt][:], w_dram["o"][kt * 128:(kt + 1) * 128, :])

    if C.max_stage <= 2:
        for pl in reversed(list(pools.values())):
            pl.release()
        return

    # ---------------- projections (emitted per head pair) ----------------
    # projection psums are 512 wide: short "s"-slot holds so the scores/exp
    # rotation stalls at most ~1.7us when a proj tile steals a slot
    def _proj_chunk(xT_t, w, dst, bias_col, ns, nw):
        ps = ps_s.tile([128, nw], F32, name="ps_kp", tag="s",
                       padded_shape=[128, PS_F])
        for kt in range(C.KT):
            nc.tensor.matmul(
                ps[:],
                w[:, kt * 128:(kt + 1) * 128],
                xT_t[kt][:, ns: ns + nw],
                start=(kt == 0), stop=(kt == C.KT - 1),
            )
        nc.vector.tensor_scalar_add(out=dst[:, ns:ns + nw], in0=ps[:],
                                    scalar1=bias_col)

    def proj_chunks(hp, kT, wk, qT, wq):
        """Closures emitting one 512-wide projection chunk each (4 K + 2 Q)."""
        out = []
        for (ns, nw) in slices(C.SK, 512):
            out.append(lambda ns=ns, nw=nw: _proj_chunk(
                xkT_t, wk, kT, bkl_sb[:, hp:hp + 1], ns, nw))
        for (ns, nw) in slices(C.SQ, 512):
            out.append(lambda ns=ns, nw=nw: _proj_chunk(
                xqT_t, wq, qT, bql_sb[:, hp:hp + 1], ns, nw))
        return out

    def k_proj(hp, kT, wk):
        for (ns, nw) in slices(C.SK, 512):
            _proj_chunk(xkT_t, wk, kT, bkl_sb[:, hp:hp + 1], ns, nw)

    def q_proj(hp, qT, wq):
        for (ns, nw) in slices(C.SQ, 512):
            _proj_chunk(xqT_t, wq, qT, bql_sb[:, hp:hp + 1], ns, nw)

    def emit_R():
        # R = bv@WoT + bo, PE-broadcast to 128 rows
        psR = ps_s.tile([1, C.DM], F32, name="psR", tag="s",
                        padded_shape=[128, PS_F])
        for (ns, nw) in slices(C.DM, 512):
            for kt in range(C.KT):
                nc.tensor.matmul(
                    psR[0:1, ns:ns + nw], bvl_sb[:, kt:kt + 1],
                    wo_t[kt][:, ns:ns + nw],
                    start=(kt == 0), stop=(kt == C.KT - 1),
                )
        Rrow = work.tile([1, C.DM], BF16, name="Rrow", tag="zrow", bufs=1,
                         padded_shape=[1, max(C.DM, C.SQS)])
        nc.vector.tensor_tensor(out=Rrow[:], in0=psR[:], in1=bo_sb[:],
                                op=ALU.add)
        psB = ps_s.tile([128, C.DM], F32, name="psB", tag="s",
                        padded_shape=[128, PS_F])
        for (ns, nw) in slices(C.DM, 512):
            nc.tensor.matmul(psB[:, ns:ns + nw], onesb[0:1, :],
                             Rrow[0:1, ns:ns + nw], start=True, stop=True)
        nc.vector.tensor_copy(R_sb[:], psB[:])

    k_proj(0, kT_buf(0), wkh_buf(0))
    q_proj(0, qT_buf(0), wqh_buf(0))

    # ---------------- attention, one head pair at a time ----------------
    for hp in range(C.HP):
        kT = kT_buf(hp)
        qT = qT_buf(hp)
        if hp + 1 < C.HP:
            load_wh(wkh_buf(hp + 1), "k", hp + 1)
            load_wh(wqh_buf(hp + 1), "q", hp + 1)
            nxt_proj = proj_chunks(hp + 1, kT_buf(hp + 1), wkh_buf(hp + 1),
                                   qT_buf(hp + 1), wqh_buf(hp + 1))
        else:
            nxt_proj = []
        for (sq, sw) in slices(C.SQ, C.SQS):
            pv = [
                ps_pv.tile([65, sw], F32, name=f"ps_pv{i}", tag="pv",
                           padded_shape=[65, PS_F])
                for i in range(2)
            ]
            PIPE = 2
            pm_hist = []

            def emit_head(j, i, hp=hp, kT=kT, qT=qT, sq=sq, sw=sw):
                """Scores MMs + exp + mask for one (j, head)."""
                ss = ps_s.tile([128, sw], F32, name=f"ps_sc{i}", tag="s",
                               padded_shape=[128, PS_F])
                for (qs, qw) in slices(sw, 512):
                    nc.tensor.matmul(
                        ss[:, qs:qs + qw],
                        kT[i * 64:(i + 1) * 64, j * 128:(j + 1) * 128],
                        qT[i * 64:(i + 1) * 64, sq + qs: sq + qs + qw],
                        start=True, stop=True,
                    )
                pe = work.tile([128, sw], BF16, name="p_exp", tag="pe",
                               bufs=2, padded_shape=[128, C.SQS])
                nc.scalar.activation(pe[:], ss[:], AF.Exp)
                pm = work.tile([128, sw], BF16, name="p_msk", tag="pm",
                               bufs=5, padded_shape=[128, C.SQS])
                nc.vector.tensor_tensor(
                    out=pm[:], in0=pe[:],
                    in1=maskT_t[j][:, sq: sq + sw],
                    op=ALU.mult,
                )
                return pm

            def emit_pv_half(jj, pmi, i, pv=pv, hp=hp, sw=sw):
                for (qs, qw) in slices(sw, 512):
                    nc.tensor.matmul(
                        pv[i][:, qs:qs + qw], v_view[:, jj, 2 * hp + i, :],
                        pmi[:, qs:qs + qw],
                        start=(jj == 0), stop=(jj == C.NJ - 1),
                    )

            def finalize_head(i, pv=pv, hp=hp, sq=sq, sw=sw):
                # copy PV rows out of PSUM immediately (DVE) and the Z row on
                # ACT, so the pv accumulator slot frees for the next head
                # pair ~3us sooner; normalize from the SBUF copies
                pvn = pv[i][0:64, :]
                zrow = work.tile([1, sw], BF16, name="zrow", tag="zrow",
                                 bufs=1, padded_shape=[1, max(C.DM, C.SQS)])
                nc.scalar.copy(zrow[0:1, :], pv[i][64:65, :])
                zb = ps_s.tile([64, sw], F32, name="zb", tag="s",
                               padded_shape=[128, PS_F])
                for (qs, qw) in slices(sw, 512):
                    nc.tensor.matmul(zb[:, qs:qs + qw], onesb[0:1, 0:64],
                                     zrow[0:1, qs:qs + qw],
                                     start=True, stop=True)
                zr = work.tile([64, sw], F32, name="zr", tag="zr", bufs=1,
                               padded_shape=[64, C.SQS])
                nc.vector.reciprocal_approx_fast(out=zr[:], in_=zb[:])
                tmp = work.tile([64, sw], BF16, name="xat_t", tag="xat_t",
                                bufs=1, padded_shape=[64, C.SQS])
                nc.vector.tensor_tensor(out=tmp[:], in0=pvn,
                                        in1=zr[:], op=ALU.mult)
                nc.sync.dma_start(
                    xattnT[64 * i:64 * (i + 1), hp * C.SQ + sq: hp * C.SQ + sq + sw],
                    tmp[:],
                )

            # software-pipelined emission, PE queue order per iteration:
            # [PV halves (deps long ready), proj chunk, scores j+1] so the
            # head-of-queue never stalls on exp's psum-slot release
            pm_hist.append((0, [emit_head(0, 0), emit_head(0, 1)]))
            for j in range(C.NJ):
                if len(pm_hist) >= PIPE:
                    jj, pp = pm_hist.pop(0)
                    emit_pv_half(jj, pp[0], 0)
                    emit_pv_half(jj, pp[1], 1)
                # overlap next head pair's projections with this attention,
                # one 512-chunk per j step so slot steals stay short
                if j >= 4 and j % 2 == 0 and nxt_proj:
                    nxt_proj.pop(0)()
                if j == 3 and hp == 1:
                    emit_R()
                if j + 1 < C.NJ:
                    pms_n = [emit_head(j + 1, 0), emit_head(j + 1, 1)]
                    pm_hist.append((j + 1, pms_n))
            # drain: finalize each head right after its last PV half so
            # head0's normalization overlaps head1's tail
            for idx, (jj, pp) in enumerate(pm_hist):
                last = idx == len(pm_hist) - 1
                emit_pv_half(jj, pp[0], 0)
                if last:
                    finalize_head(0)
                emit_pv_half(jj, pp[1], 1)
                if last:
                    finalize_head(1)

    if C.max_stage <= 3:
        for pl in reversed(list(pools.values())):
            pl.release()
        return

    # ---------------- epilogue: output projection ----------------
    for m in range(C.SQ // 128):
        ps = ps_pv.tile([128, C.DM], F32, name="ps_o", tag="pv",
                        padded_shape=[128, PS_F])
        for (qs, qw) in slices(C.DM, 512):
            for hp in range(C.HP):
                nc.tensor.matmul(
                    ps[:, qs:qs + qw],
                    xattnT[:, hp * C.SQ + m * 128: hp * C.SQ + (m + 1) * 128],
                    wo_t[hp][:, qs:qs + qw],
                    start=(hp == 0), stop=(hp == C.HP - 1),
                )
        ot = work.tile([128, C.DM], BF16, name="out_sb", tag="out_sb", bufs=2,
                       padded_shape=[128, PS_F])
        nc.vector.tensor_tensor(out=ot[:], in0=ps[:], in1=R_sb[:], op=ALU.add)
        # SWDGE cast-DMA bf16 -> f32 (Pool is idle in the epilogue)
        nc.gpsimd.dma_start(out[m * 128:(m + 1) * 128, :], ot[:])

    for pl in reversed(list(pools.values())):
        pl.release()


def build(cfg, reps=1):
    nc = bacc.Bacc("TRN2", target_bir_lowering=False, debug=False)
    C = cfg
    io = {
        "xq": nc.dram_tensor("xq", [C.SQ, C.DM], BF16, kind="ExternalInput").ap(),
        "xk": nc.dram_tensor("xk", [C.SK, C.DM], BF16, kind="ExternalInput").ap(),
        "xv": nc.dram_tensor("xv", [C.SK, C.DM], BF16, kind="ExternalInput").ap(),
        "mask": nc.dram_tensor("mask", [C.SQ, C.SK], BF16, kind="ExternalInput").ap(),
        "wqt": nc.dram_tensor("wqt", [C.DM, C.DM], BF16, kind="ExternalInput").ap(),
        "wkt": nc.dram_tensor("wkt", [C.DM, C.DM], BF16, kind="ExternalInput").ap(),
        "wvt": nc.dram_tensor("wvt", [C.DM, C.DM], BF16, kind="ExternalInput").ap(),
        "wot": nc.dram_tensor("wot", [C.DM, C.DM], BF16, kind="ExternalInput").ap(),
        "bql": nc.dram_tensor("bql", [128, C.HP], F32, kind="ExternalInput").ap(),
        "bkl": nc.dram_tensor("bkl", [128, C.HP], F32, kind="ExternalInput").ap(),
        "bvl": nc.dram_tensor("bvl", [128, C.KT], BF16, kind="ExternalInput").ap(),
        "bo_row": nc.dram_tensor("bo_row", [1, C.DM], BF16, kind="ExternalInput").ap(),
        "out": nc.dram_tensor("out", [C.SQ, C.DM], F32, kind="ExternalOutput").ap(),
    }
    with tile.TileContext(nc) as tc:
        for _ in range(reps):
            emit_kernel(tc, cfg, io)
    nc.compile()
    return nc


def host_prep(query, key, value, mask, Wq, bq, Wk, bk, Wv, bv, Wo, bo, cfg):
    """Host-side layout prep (weight transpose/cast, bf16 casts, slicing)."""
    C = cfg
    bf = ml_dtypes.bfloat16
    wqt = np.ascontiguousarray((Wq.T * 0.125).astype(bf))   # 1/sqrt(dk) folded
    wkt = np.ascontiguousarray(Wk.T.astype(bf))
    wvt = np.ascontiguousarray(Wv.T.astype(bf))
    wot = np.ascontiguousarray(Wo.T.astype(bf))
    bql = np.ascontiguousarray((bq * 0.125).reshape(C.HP, 128).T.astype(np.float32))
    bkl = np.ascontiguousarray(bk.reshape(C.HP, 128).T.astype(np.float32))
    bvl = np.ascontiguousarray(bv.reshape(C.KT, 128).T.astype(bf))
    bo_row = np.ascontiguousarray(bo.reshape(1, C.DM).astype(bf))
    shared = dict(wqt=wqt, wkt=wkt, wvt=wvt, wot=wot, bql=bql, bkl=bkl,
                  bvl=bvl, bo_row=bo_row)
    in_maps = []
    B = query.shape[0]
    halves = query.shape[1] // C.SQ
    key_bf = [np.ascontiguousarray(key[b].astype(bf)) for b in range(B)]
    val_bf = [np.ascontiguousarray(value[b].astype(bf)) for b in range(B)]
    for c in range(B * halves):
        b, h = divmod(c, halves)
        m = dict(shared)
        m["xq"] = np.ascontiguousarray(
            query[b, h * C.SQ:(h + 1) * C.SQ, :].astype(bf))
        m["xk"] = key_bf[b]
        m["xv"] = val_bf[b]
        m["mask"] = np.ascontiguousarray(
            mask[b, h * C.SQ:(h + 1) * C.SQ, :].astype(bf))
        in_maps.append(m)
    return in_maps


_CACHED = {}


def get_built():
    if "nc" not in _CACHED:
        _CACHED["nc"] = build(Cfg())
    return _CACHED["nc"]


def kernel(query, key, value, mask, Wq, bq, Wk, bk, Wv, bv, Wo, bo):
    cfg = Cfg()
    nc = get_built()
    in_maps = host_prep(query, key, value, mask, Wq, bq, Wk, bk, Wv, bv, Wo, bo, cfg)
    res = run_bass_kernel_spmd(nc, in_maps, core_ids=list(range(N_CORES)))
    B, S, DM = query.shape
    out = np.empty((B, S, DM), np.float32)
    for c in range(N_CORES):
        b, h = divmod(c, 2)
        out[b, h * cfg.SQ:(h + 1) * cfg.SQ, :] = res.results[c]["out"]
    return out
